# revision 5
# baseline (speedup 1.0000x reference)
"""MultiHeadDifferentialAttention on 8 Trainium2 NeuronCores.

Sharding: tensor-parallel over heads — core c computes heads 2c, 2c+1 for both
batch elements (full attention over T for its heads), producing the channel
slice out[:, :, 128c:128(c+1)] of the pre-LayerNorm concat.  LayerNorm needs
full-channel moments per token, so each core contributes per-token partial
(sum, sum_sq) over its 128 channels and a 32KB AllReduce(add) over all 8 cores
yields the full moments; each core then normalizes its own channel slice.
Host-side: x is pre-transposed to x^T [C, B*T] (the PE contracts along the
partition axis, so x must enter with C on partitions), weights are packed per
head pair, and the final [B*T, 128] slices are concatenated channel-wise.

Attention math per (b, h): out = softmax(q1 k1^T/8) v - lamb*softmax(q2 k2^T/8) v.
Scores are computed transposed (S^T = K Q^T, [t_k, t_q]) so exp(S^T) tiles feed
the AV matmul directly as the moving operand with t_k on partitions (no giant
transposes).  Softmax skips max-subtraction: scores are ~N(0,1) here, so exp is
safe in fp32.  The denominator rides along in the AV matmul: the stationary
operand is [V_h | ones] ([t_k 128, 64+64]), so PSUM rows 0-63 accumulate
(E V)^T and rows 64-127 accumulate sum_tk(E) replicated — the divide is then a
plain lane-wise DVE op.  (1-lamb)*gamma and (1-lamb)*beta are folded host-side.
"""
import os
import numpy as np
from contextlib import ExitStack

import concourse.bass as bass
import concourse.mybir as mybir
import concourse.tile as tile
from concourse.bass_utils import run_bass_kernel_spmd
from concourse.masks import make_identity

N_CORES = 8
B, T, C, H = 2, 2048, 1024, 16
HS = C // H                      # 64
HPC = H // N_CORES               # heads per core = 2
CS = HPC * HS                    # channel slice per core = 128
BT = B * T                       # 4096
NT = T // 128                    # 16 t_k tiles per b
NQ = T // 1024                   # 2 t_q chunks of 1024 per b
EPS = 1e-5

# matmul input dtype: float32r (fast, ~1e-4 rounded) or float32 (exact, 4x slower)
MM_DTYPE = {
    "fp32r": mybir.dt.float32r,
    "fp32": mybir.dt.float32,
}[os.environ.get("BASS_MM_DTYPE", "fp32r")]

_uid = [0]


def _legalize_waits(nc):
    """Split multi-wait instructions into 1-wait NoOps + instruction.

    The walrus build here accepts one sync-wait command per instruction, but
    TileContext emits instructions carrying several (notably its kernel-tail
    drain).  Engine-queue instructions execute in order, so hoisting extra
    waits onto same-engine NoOps right before is semantics-preserving.
    """
    for fn in nc.m.functions:
        for bb in fn.blocks:
            insts = list(bb.instructions)
            out = []
            changed = False
            for ins in insts:
                si = getattr(ins, "sync_info", None)
                waits = list(si.on_wait) if si is not None and si.on_wait else []
                if len(waits) > 1:
                    changed = True
                    for w in waits[:-1]:
                        _uid[0] += 1
                        out.append(mybir.InstNoOp(
                            name=f"I-waitsplit-{_uid[0]}",
                            sync_info=mybir.SyncInfo(on_wait=[w], on_update=[]),
                            bass_nofuse=True,
                            engine=ins.engine,
                        ))
                    ins.sync_info = mybir.SyncInfo(
                        on_wait=[waits[-1]], on_update=list(si.on_update or [])
                    )
                out.append(ins)
            if changed:
                bb.instructions = out


def _build(lamb: float):
    f32 = mybir.dt.float32
    mmdt = MM_DTYPE
    nc = bass.Bass(num_devices=N_CORES)

    xt_d = nc.declare_dram_parameter("xt", [C, BT], f32, isOutput=False)
    w_d = nc.declare_dram_parameter("wp", [5, C, CS], f32, isOutput=False)
    g_d = nc.declare_dram_parameter("gm", [CS], f32, isOutput=False)
    b_d = nc.declare_dram_parameter("bt", [CS], f32, isOutput=False)
    out_d = nc.declare_dram_parameter("out", [BT, CS], f32, isOutput=True)

    xt3 = xt_d.ap().rearrange("(k p) t -> p k t", p=128)          # [128, 8, 4096]
    w4 = w_d.ap().rearrange("w (k p) m -> w k p m", p=128)        # [5, 8, 128, 128]

    with tile.TileContext(nc) as tc, ExitStack() as ctx:
        const = ctx.enter_context(tc.tile_pool(name="const", bufs=1))
        sbx = ctx.enter_context(tc.tile_pool(name="sbx", bufs=2))
        sbqk = ctx.enter_context(tc.tile_pool(name="sbqk", bufs=1))
        sbe = ctx.enter_context(tc.tile_pool(name="sbe", bufs=3))
        sbn = ctx.enter_context(tc.tile_pool(name="sbn", bufs=2))
        sbo = ctx.enter_context(tc.tile_pool(name="sbo", bufs=2))
        ps_a = ctx.enter_context(tc.tile_pool(name="ps_a", bufs=2, space="PSUM"))
        ps_s = ctx.enter_context(tc.tile_pool(name="ps_s", bufs=2, space="PSUM"))
        ps_o = ctx.enter_context(tc.tile_pool(name="ps_o", bufs=1, space="PSUM"))
        dram = ctx.enter_context(tc.tile_pool(name="dram", bufs=1, space="DRAM"))

        # ---- constants ----
        ident = const.tile([128, 128], f32, tag="ident")
        make_identity(nc, ident)
        gamma = const.tile([128, CS], f32, tag="gamma")
        beta = const.tile([128, CS], f32, tag="beta")
        nc.sync.dma_start(out=gamma, in_=g_d.ap().partition_broadcast(128))
        nc.sync.dma_start(out=beta, in_=b_d.ap().partition_broadcast(128))
        eps_t = const.tile([128, 1], f32, tag="eps")
        nc.vector.memset(eps_t, EPS)

        # weights: 5 proj x 8 k-tiles, each [128 c, 128 m]
        w_sb = []
        for p5 in range(5):
            row = []
            for k in range(8):
                wt = const.tile([128, 128], mmdt, tag=f"w{p5}{k}", name=f"w{p5}{k}")
                nc.sync.dma_start(out=wt, in_=w4[p5, k].bitcast(mmdt))
                row.append(wt)
            w_sb.append(row)

        # AV stationary tiles [t_k 128, 64 V | 64 ones] per (head, t_k tile)
        avw = [[const.tile([128, 128], mmdt, tag=f"avw{h}{i}", name=f"avw{h}{i}")
                for i in range(NT)] for h in range(HPC)]
        ones_t = const.tile([128, HS], f32, tag="ones_t")
        nc.vector.memset(ones_t, 1.0)
        for h in range(HPC):
            for i in range(NT):
                nc.vector.tensor_copy(avw[h][i][:, HS:128], ones_t[:, :])

        # persistent per-b projection buffers [128, T]
        qk = [sbqk.tile([128, T], mmdt, tag=f"qk{w}", name=f"qk{w}") for w in range(4)]
        vT = sbqk.tile([128, T], f32, tag="vT")
        stack = sbqk.tile([128, T], f32, tag="stack")  # combined heads, pre-transpose
        preln = sbqk.tile([128, BT], f32, tag="preln")  # [t 128, 32 tiles x 128 chan]
        stats = const.tile([128, 2 * (BT // 128)], f32, tag="stats")
        sq_scr = const.tile([128, 128], f32, tag="sq_scr")

        pre3 = preln.rearrange("p (i c) -> p i c", c=128)

        for b in range(B):
            # ---- projections: q1,k1,q2,k2 -> qk[w] (transposed [2h*hs, T]), v -> vT
            for ch in range(4):                       # 512-token chunks
                xt_sb = sbx.tile([128, 8, 512], mmdt, tag="xt")
                col0 = b * T + ch * 512
                nc.sync.dma_start(out=xt_sb, in_=xt3[:, :, col0:col0 + 512].bitcast(mmdt))
                for p5 in range(5):
                    pp = ps_a.tile([128, 512], f32, tag="pp")
                    for k in range(8):
                        nc.tensor.matmul(pp[:, :], w_sb[p5][k][:, :], xt_sb[:, k, :],
                                         start=(k == 0), stop=(k == 7))
                    dst = qk[p5] if p5 < 4 else vT
                    nc.vector.tensor_copy(dst[:, ch * 512:(ch + 1) * 512], pp[:, :])

            # ---- V^T -> V tiles into avw[h][i][:, 0:64]
            for i in range(NT):
                pt = ps_a.tile([128, 128], f32, tag="pp")
                nc.tensor.transpose(pt[:, :], vT[:, i * 128:(i + 1) * 128], ident[:, :])
                for h in range(HPC):
                    nc.vector.tensor_copy(avw[h][i][:, 0:HS], pt[:, h * HS:(h + 1) * HS])

            # ---- attention per (h, qc, type)
            for h in range(HPC):
                hp = h * HS
                for qc in range(NQ):
                    q0 = qc * 1024
                    norm1 = sbn.tile([HS, 1024], f32, tag="norm1")
                    for ty in range(2):
                        qb, kb = qk[2 * ty], qk[2 * ty + 1]
                        po = ps_o.tile([128, 1024], f32, tag="po")
                        for tk in range(NT):
                            sS = ps_s.tile([128, 1024], f32, tag="sS")
                            for half in range(2):
                                nc.tensor.matmul(
                                    sS[:, half * 512:(half + 1) * 512],
                                    kb[hp:hp + HS, tk * 128:(tk + 1) * 128],
                                    qb[hp:hp + HS, q0 + half * 512:q0 + (half + 1) * 512],
                                    start=True, stop=True)
                            eT = sbe.tile([128, 1024], mmdt, tag="eT")
                            nc.scalar.activation(out=eT[:, :], in_=sS[:, :],
                                                 func=mybir.ActivationFunctionType.Exp,
                                                 scale=0.125)
                            for half in range(2):
                                nc.tensor.matmul(
                                    po[:, half * 512:(half + 1) * 512],
                                    avw[h][tk][:, :],
                                    eT[:, half * 512:(half + 1) * 512],
                                    start=(tk == 0), stop=(tk == NT - 1))
                        # normalize: rows 0:64 = (E V)^T, rows 64:128 = den
                        rcp = sbn.tile([HS, 1024], f32, tag="rcp")
                        nc.vector.reciprocal(rcp[:, :], po[HS:128, :])
                        if ty == 0:
                            nc.vector.tensor_mul(norm1[:, :], po[0:HS, :], rcp[:, :])
                        else:
                            t2 = sbn.tile([HS, 1024], f32, tag="t2")
                            nc.vector.tensor_mul(t2[:, :], po[0:HS, :], rcp[:, :])
                            nc.vector.scalar_tensor_tensor(
                                out=stack[hp:hp + HS, q0:q0 + 1024],
                                in0=t2[:, :], scalar=-lamb, in1=norm1[:, :],
                                op0=mybir.AluOpType.mult, op1=mybir.AluOpType.add)

            # ---- transpose combined -> [t, chan], moment partials
            for i in range(NT):
                gi = b * NT + i
                pt = ps_a.tile([128, 128], f32, tag="pp")
                nc.tensor.transpose(pt[:, :], stack[:, i * 128:(i + 1) * 128], ident[:, :])
                nc.vector.tensor_scalar(
                    out=pre3[:, gi, :], in0=pt[:, :], scalar1=0.0, scalar2=1.0,
                    op0=mybir.AluOpType.add, op1=mybir.AluOpType.mult,
                    accum_out=stats[:, 2 * gi:2 * gi + 1])
                nc.scalar.activation(out=sq_scr[:, :], in_=pt[:, :],
                                     func=mybir.ActivationFunctionType.Square,
                                     accum_out=stats[:, 2 * gi + 1:2 * gi + 2])

        # ---- AllReduce per-token moments across the 8 cores
        cc_in = dram.tile([128, 2 * (BT // 128)], f32)
        cc_out = dram.tile([128, 2 * (BT // 128)], f32)
        nc.sync.dma_start(out=cc_in[:, :], in_=stats[:, :])
        nc.gpsimd.collective_compute(
            "AllReduce", mybir.AluOpType.add,
            replica_groups=[list(range(N_CORES))],
            ins=[cc_in.opt()], outs=[cc_out.opt()])
        statsf = const.tile([128, 2 * (BT // 128)], f32, tag="statsf")
        nc.sync.dma_start(out=statsf[:, :], in_=cc_out[:, :])

        # ---- moments -> mean, rstd  [128, 32]
        ntile = BT // 128
        sf3 = statsf.rearrange("p (i two) -> p i two", two=2)
        mean = const.tile([128, ntile], f32, tag="mean")
        rstd = const.tile([128, ntile], f32, tag="rstd")
        var = const.tile([128, ntile], f32, tag="var")
        nc.vector.tensor_scalar_mul(mean[:, :], sf3[:, :, 0], 1.0 / C)
        nc.vector.tensor_scalar_mul(var[:, :], sf3[:, :, 1], 1.0 / C)
        msq = const.tile([128, ntile], f32, tag="msq")
        nc.vector.tensor_mul(msq[:, :], mean[:, :], mean[:, :])
        nc.vector.tensor_sub(var[:, :], var[:, :], msq[:, :])
        nc.scalar.activation(out=var[:, :], in_=var[:, :],
                             func=mybir.ActivationFunctionType.Sqrt,
                             bias=eps_t[:, :], scale=1.0)
        nc.vector.reciprocal(rstd[:, :], var[:, :])

        # ---- apply LN + folded (1-lamb)*gamma/beta, store slice
        for gi in range(ntile):
            o1 = sbo.tile([128, CS], f32, tag="o1")
            nc.vector.tensor_scalar(
                out=o1[:, :], in0=pre3[:, gi, :],
                scalar1=mean[:, gi:gi + 1], scalar2=rstd[:, gi:gi + 1],
                op0=mybir.AluOpType.subtract, op1=mybir.AluOpType.mult)
            o2 = sbo.tile([128, CS], f32, tag="o2")
            nc.vector.tensor_mul(o2[:, :], o1[:, :], gamma[:, :])
            nc.vector.tensor_add(o2[:, :], o2[:, :], beta[:, :])
            nc.sync.dma_start(out=out_d[gi * 128:(gi + 1) * 128, :], in_=o2[:, :])

    _legalize_waits(nc)
    return nc


_cache = {}


def _get_nc(lamb: float):
    key = (round(lamb, 9), str(MM_DTYPE))
    if key not in _cache:
        _cache[key] = _build(lamb)
    return _cache[key]


def kernel(x, wq1, wk1, wq2, wk2, wv, ln_gamma, ln_beta, lamb):
    x = np.asarray(x, dtype=np.float32)
    lam = float(np.asarray(lamb))
    xt = np.ascontiguousarray(x.reshape(BT, C).T)          # [C, BT]
    g = np.asarray(ln_gamma, np.float32) * (1.0 - lam)
    bt = np.asarray(ln_beta, np.float32) * (1.0 - lam)

    nc = _get_nc(lam)
    in_maps = []
    for c in range(N_CORES):
        h0 = c * HPC
        wp = np.stack([
            np.concatenate([np.asarray(w, np.float32)[h0 + j] for j in range(HPC)], axis=1)
            for w in (wq1, wk1, wq2, wk2, wv)
        ])                                                  # [5, C, 128]
        in_maps.append({
            "xt": xt,
            "wp": np.ascontiguousarray(wp),
            "gm": np.ascontiguousarray(g[c * CS:(c + 1) * CS]),
            "bt": np.ascontiguousarray(bt[c * CS:(c + 1) * CS]),
        })

    res = run_bass_kernel_spmd(nc, in_maps, list(range(N_CORES)))
    full = np.concatenate([res.results[c]["out"] for c in range(N_CORES)], axis=1)
    return full.reshape(B, T, C)


# revision 10
# speedup vs baseline: 1.0847x; 1.0847x over previous
"""MultiHeadDifferentialAttention on 8 Trainium2 NeuronCores.

Sharding: tensor-parallel over heads — core c computes heads 2c, 2c+1 for both
batch elements (full attention over T for its heads), producing the channel
slice out[:, :, 128c:128(c+1)] of the pre-LayerNorm concat.  LayerNorm needs
full-channel moments per token, so each core contributes per-token partial
(sum, sum_sq) over its 128 channels and a 32KB AllReduce(add) over all 8 cores
yields the full moments; each core then normalizes its own channel slice.
Host-side: x is pre-transposed to x^T [C, B*T] (the PE contracts along the
partition axis, so x must enter with C on partitions), weights are packed per
head pair, and the final [B*T, 128] slices are concatenated channel-wise.

Attention math per (b, h): out = softmax(q1 k1^T/8) v - lamb*softmax(q2 k2^T/8) v.
Scores are computed transposed (S^T = K Q^T, [t_k, t_q]) so exp(S^T) tiles feed
the AV matmul directly as the moving operand with t_k on partitions (no giant
transposes).  Softmax skips max-subtraction: scores are ~N(0,1) here, so exp is
safe in fp32.  The denominator rides along in the AV matmul: the stationary
operand is [V_h | ones] ([t_k 128, 64+64]), so PSUM rows 0-63 accumulate
(E V)^T and rows 64-127 accumulate sum_tk(E) replicated — the divide is then a
plain lane-wise DVE op.  (1-lamb)*gamma and (1-lamb)*beta are folded host-side.
"""
import os
import numpy as np
from contextlib import ExitStack

import concourse.bass as bass
import concourse.mybir as mybir
import concourse.tile as tile
from concourse.bass_utils import run_bass_kernel_spmd
from concourse.masks import make_identity

N_CORES = 8
B, T, C, H = 2, 2048, 1024, 16
HS = C // H                      # 64
HPC = H // N_CORES               # heads per core = 2
CS = HPC * HS                    # channel slice per core = 128
BT = B * T                       # 4096
NT = T // 128                    # 16 t_k tiles per b
NQ = T // 1024                   # 2 t_q chunks of 1024 per b
EPS = 1e-5

# matmul input dtype: float32r (fast, ~1e-4 rounded) or float32 (exact, 4x slower)
MM_DTYPE = {
    "fp32r": mybir.dt.float32r,
    "fp32": mybir.dt.float32,
}[os.environ.get("BASS_MM_DTYPE", "fp32r")]

_uid = [0]


def _legalize_waits(nc):
    """Split multi-wait instructions into 1-wait NoOps + instruction.

    The walrus build here accepts one sync-wait command per instruction, but
    TileContext emits instructions carrying several (notably its kernel-tail
    drain).  Engine-queue instructions execute in order, so hoisting extra
    waits onto same-engine NoOps right before is semantics-preserving.
    """
    for fn in nc.m.functions:
        for bb in fn.blocks:
            insts = list(bb.instructions)
            out = []
            changed = False
            for ins in insts:
                si = getattr(ins, "sync_info", None)
                waits = list(si.on_wait) if si is not None and si.on_wait else []
                if len(waits) > 1:
                    changed = True
                    for w in waits[:-1]:
                        _uid[0] += 1
                        out.append(mybir.InstNoOp(
                            name=f"I-waitsplit-{_uid[0]}",
                            sync_info=mybir.SyncInfo(on_wait=[w], on_update=[]),
                            bass_nofuse=True,
                            engine=ins.engine,
                        ))
                    ins.sync_info = mybir.SyncInfo(
                        on_wait=[waits[-1]], on_update=list(si.on_update or [])
                    )
                out.append(ins)
            if changed:
                bb.instructions = out


def _build(lamb: float):
    f32 = mybir.dt.float32
    mmdt = MM_DTYPE
    nc = bass.Bass(num_devices=N_CORES)

    xt_d = nc.declare_dram_parameter("xt", [C, BT], f32, isOutput=False)
    w_d = nc.declare_dram_parameter("wp", [5, C, CS], f32, isOutput=False)
    g_d = nc.declare_dram_parameter("gm", [CS], f32, isOutput=False)
    b_d = nc.declare_dram_parameter("bt", [CS], f32, isOutput=False)
    out_d = nc.declare_dram_parameter("out", [BT, CS], f32, isOutput=True)
    debug = bool(int(os.environ.get("BASS_DEBUG_DUMPS", "0")))
    if debug:
        dbg_qk = nc.declare_dram_parameter("dbg_qk", [4, 128, T], f32, isOutput=True)
        dbg_vt = nc.declare_dram_parameter("dbg_vt", [128, T], f32, isOutput=True)
        dbg_stack = nc.declare_dram_parameter("dbg_stack", [128, T], f32, isOutput=True)
        dbg_stats = nc.declare_dram_parameter("dbg_stats", [128, 2 * (BT // 128)], f32, isOutput=True)
        dbg_statsf = nc.declare_dram_parameter("dbg_statsf", [128, 2 * (BT // 128)], f32, isOutput=True)

    xt3 = xt_d.ap().rearrange("(k p) t -> p k t", p=128)          # [128, 8, 4096]
    w4 = w_d.ap().rearrange("w (k p) m -> w k p m", p=128)        # [5, 8, 128, 128]

    with tile.TileContext(nc) as tc, ExitStack() as ctx:
        const = ctx.enter_context(tc.tile_pool(name="const", bufs=1))
        sbx = ctx.enter_context(tc.tile_pool(name="sbx", bufs=2))
        sbqk = ctx.enter_context(tc.tile_pool(name="sbqk", bufs=1))
        sbe = ctx.enter_context(tc.tile_pool(name="sbe", bufs=3))
        sbn = ctx.enter_context(tc.tile_pool(name="sbn", bufs=2))
        sbo = ctx.enter_context(tc.tile_pool(name="sbo", bufs=2))
        ps_a = ctx.enter_context(tc.tile_pool(name="ps_a", bufs=2, space="PSUM"))
        ps_s = ctx.enter_context(tc.tile_pool(name="ps_s", bufs=2, space="PSUM"))
        ps_o = ctx.enter_context(tc.tile_pool(name="ps_o", bufs=1, space="PSUM"))
        dram = ctx.enter_context(tc.tile_pool(name="dram", bufs=1, space="DRAM"))

        # ---- constants ----
        ident = const.tile([128, 128], f32, tag="ident")
        make_identity(nc, ident)
        gamma = const.tile([128, CS], f32, tag="gamma")
        beta = const.tile([128, CS], f32, tag="beta")
        nc.sync.dma_start(out=gamma, in_=g_d.ap().partition_broadcast(128))
        nc.sync.dma_start(out=beta, in_=b_d.ap().partition_broadcast(128))
        eps_t = const.tile([128, 1], f32, tag="eps")
        nc.vector.memset(eps_t, EPS)

        # weights: 5 proj x 8 k-tiles, each [128 c, 128 m]
        w_sb = []
        for p5 in range(5):
            row = []
            for k in range(8):
                wt = const.tile([128, 128], mmdt, tag=f"w{p5}{k}", name=f"w{p5}{k}")
                nc.sync.dma_start(out=wt, in_=w4[p5, k].bitcast(mmdt))
                row.append(wt)
            w_sb.append(row)

        # AV stationary tiles [t_k 128, 64 V | 64 ones] per (head, t_k tile)
        avw = [[const.tile([128, 128], mmdt, tag=f"avw{h}{i}", name=f"avw{h}{i}")
                for i in range(NT)] for h in range(HPC)]
        ones_t = const.tile([128, HS], f32, tag="ones_t")
        nc.vector.memset(ones_t, 1.0)
        for h in range(HPC):
            for i in range(NT):
                nc.vector.tensor_copy(avw[h][i][:, HS:128], ones_t[:, :])

        # persistent per-b projection buffers [128, T]
        qk = [sbqk.tile([128, T], mmdt, tag=f"qk{w}", name=f"qk{w}") for w in range(4)]
        vT = sbqk.tile([128, T], f32, tag="vT")
        stack = sbqk.tile([128, T], f32, tag="stack")  # combined heads, pre-transpose
        preln = sbqk.tile([128, BT], f32, tag="preln")  # [t 128, 32 tiles x 128 chan]
        stats = const.tile([128, 2 * (BT // 128)], f32, tag="stats")
        sq_scr = const.tile([128, 128], f32, tag="sq_scr")

        pre3 = preln.rearrange("p (i c) -> p i c", c=128)

        for b in range(B):
            # ---- projections: q1,k1,q2,k2 -> qk[w] (transposed [2h*hs, T]), v -> vT
            for ch in range(4):                       # 512-token chunks
                xt_sb = sbx.tile([128, 8, 512], mmdt, tag="xt")
                col0 = b * T + ch * 512
                nc.sync.dma_start(out=xt_sb, in_=xt3[:, :, col0:col0 + 512].bitcast(mmdt))
                for p5 in range(5):
                    pp = ps_a.tile([128, 512], f32, tag="pp")
                    for k in range(8):
                        nc.tensor.matmul(pp[:, :], w_sb[p5][k][:, :], xt_sb[:, k, :],
                                         start=(k == 0), stop=(k == 7))
                    dst = qk[p5] if p5 < 4 else vT
                    nc.vector.tensor_copy(dst[:, ch * 512:(ch + 1) * 512], pp[:, :])

            # ---- V^T -> V tiles into avw[h][i][:, 0:64]
            for i in range(NT):
                pt = ps_a.tile([128, 128], f32, tag="pp")
                nc.tensor.transpose(pt[:, :], vT[:, i * 128:(i + 1) * 128], ident[:, :])
                for h in range(HPC):
                    nc.vector.tensor_copy(avw[h][i][:, 0:HS], pt[:, h * HS:(h + 1) * HS])

            # ---- attention per (h, qc, type)
            for h in range(HPC):
                hp = h * HS
                for qc in range(NQ):
                    q0 = qc * 1024
                    norm1 = sbn.tile([HS, 1024], f32, tag="norm1")
                    for ty in range(2):
                        qb, kb = qk[2 * ty], qk[2 * ty + 1]
                        po = ps_o.tile([128, 1024], f32, tag="po")
                        for tk in range(NT):
                            sS = ps_s.tile([128, 1024], f32, tag="sS")
                            for half in range(2):
                                nc.tensor.matmul(
                                    sS[:, half * 512:(half + 1) * 512],
                                    kb[hp:hp + HS, tk * 128:(tk + 1) * 128],
                                    qb[hp:hp + HS, q0 + half * 512:q0 + (half + 1) * 512],
                                    start=True, stop=True)
                            eT = sbe.tile([128, 1024], mmdt, tag="eT")
                            nc.scalar.activation(out=eT[:, :], in_=sS[:, :],
                                                 func=mybir.ActivationFunctionType.Exp,
                                                 scale=0.125)
                            for half in range(2):
                                nc.tensor.matmul(
                                    po[:, half * 512:(half + 1) * 512],
                                    avw[h][tk][:, :],
                                    eT[:, half * 512:(half + 1) * 512],
                                    start=(tk == 0), stop=(tk == NT - 1))
                        # normalize: rows 0:64 = (E V)^T, rows 64:128 = den
                        rcp = sbn.tile([HS, 1024], f32, tag="rcp")
                        nc.vector.reciprocal(rcp[:, :], po[HS:128, :])
                        if ty == 0:
                            nc.vector.tensor_mul(norm1[:, :], po[0:HS, :], rcp[:, :])
                        else:
                            t2 = sbn.tile([HS, 1024], f32, tag="t2")
                            nc.vector.tensor_mul(t2[:, :], po[0:HS, :], rcp[:, :])
                            nc.vector.scalar_tensor_tensor(
                                out=stack[hp:hp + HS, q0:q0 + 1024],
                                in0=t2[:, :], scalar=-lamb, in1=norm1[:, :],
                                op0=mybir.AluOpType.mult, op1=mybir.AluOpType.add)

            if debug and b == 0:
                for w in range(4):
                    nc.sync.dma_start(out=dbg_qk[w], in_=qk[w][:, :].bitcast(f32))
                nc.sync.dma_start(out=dbg_vt[:, :], in_=vT[:, :])
                nc.sync.dma_start(out=dbg_stack[:, :], in_=stack[:, :])

            # ---- transpose combined -> [t, chan], moment partials
            for i in range(NT):
                gi = b * NT + i
                pt = ps_a.tile([128, 128], f32, tag="pp")
                nc.tensor.transpose(pt[:, :], stack[:, i * 128:(i + 1) * 128], ident[:, :])
                nc.vector.tensor_scalar(
                    out=pre3[:, gi, :], in0=pt[:, :], scalar1=0.0, scalar2=0.0,
                    op0=mybir.AluOpType.add, op1=mybir.AluOpType.add,
                    accum_out=stats[:, 2 * gi:2 * gi + 1])
                nc.scalar.activation(out=sq_scr[:, :], in_=pt[:, :],
                                     func=mybir.ActivationFunctionType.Square,
                                     accum_out=stats[:, 2 * gi + 1:2 * gi + 2])

        # ---- AllReduce per-token moments across the 8 cores
        cc_in = dram.tile([128, 2 * (BT // 128)], f32)
        cc_out = dram.tile([128, 2 * (BT // 128)], f32)
        nc.sync.dma_start(out=cc_in[:, :], in_=stats[:, :])
        nc.gpsimd.collective_compute(
            "AllReduce", mybir.AluOpType.add,
            replica_groups=[list(range(N_CORES))],
            ins=[cc_in.opt()], outs=[cc_out.opt()])
        statsf = const.tile([128, 2 * (BT // 128)], f32, tag="statsf")
        nc.sync.dma_start(out=statsf[:, :], in_=cc_out[:, :])
        if debug:
            nc.sync.dma_start(out=dbg_stats[:, :], in_=stats[:, :])
            nc.sync.dma_start(out=dbg_statsf[:, :], in_=statsf[:, :])

        # ---- moments -> mean, rstd  [128, 32]
        ntile = BT // 128
        sf3 = statsf.rearrange("p (i two) -> p i two", two=2)
        mean = const.tile([128, ntile], f32, tag="mean")
        rstd = const.tile([128, ntile], f32, tag="rstd")
        var = const.tile([128, ntile], f32, tag="var")
        nc.vector.tensor_scalar_mul(mean[:, :], sf3[:, :, 0], 1.0 / C)
        nc.vector.tensor_scalar_mul(var[:, :], sf3[:, :, 1], 1.0 / C)
        msq = const.tile([128, ntile], f32, tag="msq")
        nc.vector.tensor_mul(msq[:, :], mean[:, :], mean[:, :])
        nc.vector.tensor_sub(var[:, :], var[:, :], msq[:, :])
        nc.scalar.activation(out=var[:, :], in_=var[:, :],
                             func=mybir.ActivationFunctionType.Sqrt,
                             bias=eps_t[:, :], scale=1.0)
        nc.vector.reciprocal(rstd[:, :], var[:, :])

        # ---- apply LN + folded (1-lamb)*gamma/beta, store slice
        for gi in range(ntile):
            o1 = sbo.tile([128, CS], f32, tag="o1")
            nc.vector.tensor_scalar(
                out=o1[:, :], in0=pre3[:, gi, :],
                scalar1=mean[:, gi:gi + 1], scalar2=rstd[:, gi:gi + 1],
                op0=mybir.AluOpType.subtract, op1=mybir.AluOpType.mult)
            o2 = sbo.tile([128, CS], f32, tag="o2")
            nc.vector.tensor_mul(o2[:, :], o1[:, :], gamma[:, :])
            nc.vector.tensor_add(o2[:, :], o2[:, :], beta[:, :])
            nc.sync.dma_start(out=out_d[gi * 128:(gi + 1) * 128, :], in_=o2[:, :])

    _legalize_waits(nc)
    return nc


_cache = {}


def _get_nc(lamb: float):
    key = (round(lamb, 9), str(MM_DTYPE), os.environ.get("BASS_DEBUG_DUMPS", "0"))
    if key not in _cache:
        _cache[key] = _build(lamb)
    return _cache[key]


def kernel(x, wq1, wk1, wq2, wk2, wv, ln_gamma, ln_beta, lamb):
    x = np.asarray(x, dtype=np.float32)
    lam = float(np.asarray(lamb))
    xt = np.ascontiguousarray(x.reshape(BT, C).T)          # [C, BT]
    g = np.asarray(ln_gamma, np.float32) * (1.0 - lam)
    bt = np.asarray(ln_beta, np.float32) * (1.0 - lam)

    nc = _get_nc(lam)
    in_maps = []
    for c in range(N_CORES):
        h0 = c * HPC
        wp = np.stack([
            np.concatenate([np.asarray(w, np.float32)[h0 + j] for j in range(HPC)], axis=1)
            for w in (wq1, wk1, wq2, wk2, wv)
        ])                                                  # [5, C, 128]
        in_maps.append({
            "xt": xt,
            "wp": np.ascontiguousarray(wp),
            "gm": np.ascontiguousarray(g[c * CS:(c + 1) * CS]),
            "bt": np.ascontiguousarray(bt[c * CS:(c + 1) * CS]),
        })

    res = run_bass_kernel_spmd(nc, in_maps, list(range(N_CORES)))
    full = np.concatenate([res.results[c]["out"] for c in range(N_CORES)], axis=1)
    return full.reshape(B, T, C)


# revision 12
# speedup vs baseline: 14.8017x; 13.6462x over previous
"""MultiHeadDifferentialAttention on 8 Trainium2 NeuronCores.

Sharding: tensor-parallel over heads — core c computes heads 2c, 2c+1 for both
batch elements (full attention over T for its heads), producing the channel
slice out[:, :, 128c:128(c+1)] of the pre-LayerNorm concat.  LayerNorm needs
full-channel moments per token, so each core contributes per-token partial
(sum, sum_sq) over its 128 channels and a 32KB AllReduce(add) over all 8 cores
yields the full moments; each core then normalizes its own channel slice.
Host-side: x is pre-transposed to x^T [C, B*T] (the PE contracts along the
partition axis, so x must enter with C on partitions), weights are packed per
head pair, and the final [B*T, 128] slices are concatenated channel-wise.

Attention math per (b, h): out = softmax(q1 k1^T/8) v - lamb*softmax(q2 k2^T/8) v.
Scores are computed transposed (S^T = K Q^T, [t_k, t_q]) so exp(S^T) tiles feed
the AV matmul directly as the moving operand with t_k on partitions (no giant
transposes).  Softmax skips max-subtraction: scores are ~N(0,1) here, so exp is
safe in fp32.  The denominator rides along in the AV matmul: the stationary
operand is [V_h | ones] ([t_k 128, 64+64]), so PSUM rows 0-63 accumulate
(E V)^T and rows 64-127 accumulate sum_tk(E) replicated — the divide is then a
plain lane-wise DVE op.  (1-lamb)*gamma and (1-lamb)*beta are folded host-side.
"""
import os
import numpy as np
from contextlib import ExitStack

import concourse.bass as bass
import concourse.mybir as mybir
import concourse.tile as tile
from concourse.bass_utils import run_bass_kernel_spmd
from concourse.masks import make_identity

N_CORES = 8
B, T, C, H = 2, 2048, 1024, 16
HS = C // H                      # 64
HPC = H // N_CORES               # heads per core = 2
CS = HPC * HS                    # channel slice per core = 128
BT = B * T                       # 4096
NT = T // 128                    # 16 t_k tiles per b
NQ = T // 1024                   # 2 t_q chunks of 1024 per b
NTILE = BT // 128                # 32 output row tiles
EPS = 1e-5

# matmul input dtype: float32r (fast, ~1e-4 rounded) or float32 (exact, 4x slower)
MM_DTYPE = {
    "fp32r": mybir.dt.float32r,
    "fp32": mybir.dt.float32,
}[os.environ.get("BASS_MM_DTYPE", "fp32r")]

_uid = [0]


def _legalize_waits(nc):
    """Split multi-wait instructions into 1-wait NoOps + instruction.

    The walrus build in this container accepts one sync-wait command per
    instruction, but TileContext emits instructions carrying several (notably
    its kernel-tail drain).  Engine-queue instructions execute in order, so
    hoisting extra waits onto same-engine NoOps right before is
    semantics-preserving.
    """
    for fn in nc.m.functions:
        for bb in fn.blocks:
            insts = list(bb.instructions)
            out = []
            changed = False
            for ins in insts:
                si = getattr(ins, "sync_info", None)
                waits = list(si.on_wait) if si is not None and si.on_wait else []
                if len(waits) > 1:
                    changed = True
                    for w in waits[:-1]:
                        _uid[0] += 1
                        out.append(mybir.InstNoOp(
                            name=f"I-waitsplit-{_uid[0]}",
                            sync_info=mybir.SyncInfo(on_wait=[w], on_update=[]),
                            bass_nofuse=True,
                            engine=ins.engine,
                        ))
                    ins.sync_info = mybir.SyncInfo(
                        on_wait=[waits[-1]], on_update=list(si.on_update or [])
                    )
                out.append(ins)
            if changed:
                bb.instructions = out


class _Env:
    pass


def _emit_compute(nc, e, lamb):
    """One full forward pass: projections, attention, LN. Emitted `nrep` times
    for slope-based HW timing (BASS_REPEAT)."""
    f32 = mybir.dt.float32
    mmdt = MM_DTYPE

    for b in range(B):
        # ---- projections: q1,k1,q2,k2 -> qk[w] ([2h*hs, T] transposed), v -> vT
        for ch in range(4):                       # 512-token chunks
            xt_sb = e.sbx.tile([128, 8, 512], mmdt, tag="xt", name="xt_sb")
            col0 = b * T + ch * 512
            nc.sync.dma_start(out=xt_sb, in_=e.xt3[:, :, col0:col0 + 512].bitcast(mmdt))
            for p5 in range(5):
                pp = e.ps_a.tile([128, 512], f32, tag="pp", name="pp")
                for k in range(8):
                    nc.tensor.matmul(pp[:, :], e.w_sb[p5][k][:, :], xt_sb[:, k, :],
                                     start=(k == 0), stop=(k == 7))
                dst = e.qk[p5] if p5 < 4 else e.vT
                nc.vector.tensor_copy(dst[:, ch * 512:(ch + 1) * 512], pp[:, :])

        # ---- V^T -> V tiles into avw[h][i][:, 0:64]
        for i in range(NT):
            pt = e.ps_a.tile([128, 128], f32, tag="pp", name="pt")
            nc.tensor.transpose(pt[:, :], e.vT[:, i * 128:(i + 1) * 128], e.ident[:, :])
            for h in range(HPC):
                nc.vector.tensor_copy(e.avw[h][i][:, 0:HS], pt[:, h * HS:(h + 1) * HS])

        # ---- attention per (h, qc, type)
        for h in range(HPC):
            hp = h * HS
            for qc in range(NQ):
                q0 = qc * 1024
                norm1 = e.sbn.tile([HS, 1024], f32, tag="norm1", name="norm1")
                for ty in range(2):
                    qb, kb = e.qk[2 * ty], e.qk[2 * ty + 1]
                    po = e.ps_o.tile([128, 1024], f32, tag="po", name="po")
                    for tk in range(NT):
                        sS = e.ps_s.tile([128, 1024], f32, tag="sS", name="sS")
                        for hf in range(2):
                            nc.tensor.matmul(
                                sS[:, hf * 512:(hf + 1) * 512],
                                kb[hp:hp + HS, tk * 128:(tk + 1) * 128],
                                qb[hp:hp + HS, q0 + hf * 512:q0 + (hf + 1) * 512],
                                start=True, stop=True)
                        eT = e.sbe.tile([128, 1024], mmdt, tag="eT", name="eT")
                        nc.scalar.activation(out=eT[:, :], in_=sS[:, :],
                                             func=mybir.ActivationFunctionType.Exp,
                                             scale=0.125)
                        for hf in range(2):
                            nc.tensor.matmul(
                                po[:, hf * 512:(hf + 1) * 512],
                                e.avw[h][tk][:, :],
                                eT[:, hf * 512:(hf + 1) * 512],
                                start=(tk == 0), stop=(tk == NT - 1))
                    # normalize: rows 0:64 = (E V)^T, rows 64:128 = den
                    rcp = e.sbn.tile([HS, 1024], f32, tag="rcp", name="rcp")
                    nc.vector.reciprocal(rcp[:, :], po[HS:128, :])
                    if ty == 0:
                        nc.vector.tensor_mul(norm1[:, :], po[0:HS, :], rcp[:, :])
                    else:
                        t2 = e.sbn.tile([HS, 1024], f32, tag="t2", name="t2")
                        nc.vector.tensor_mul(t2[:, :], po[0:HS, :], rcp[:, :])
                        nc.vector.scalar_tensor_tensor(
                            out=e.stack[hp:hp + HS, q0:q0 + 1024],
                            in0=t2[:, :], scalar=-lamb, in1=norm1[:, :],
                            op0=mybir.AluOpType.mult, op1=mybir.AluOpType.add)

        if e.debug and b == 0:
            for w in range(4):
                nc.sync.dma_start(out=e.dbg_qk[w], in_=e.qk[w][:, :].bitcast(f32))
            nc.sync.dma_start(out=e.dbg_vt[:, :], in_=e.vT[:, :])
            nc.sync.dma_start(out=e.dbg_stack[:, :], in_=e.stack[:, :])

        # ---- transpose combined -> [t, chan], moment partials
        for i in range(NT):
            gi = b * NT + i
            pt2 = e.ps_a.tile([128, 128], f32, tag="pp", name="pt2")
            nc.tensor.transpose(pt2[:, :], e.stack[:, i * 128:(i + 1) * 128], e.ident[:, :])
            nc.vector.tensor_scalar(
                out=e.pre3[:, gi, :], in0=pt2[:, :], scalar1=0.0, scalar2=0.0,
                op0=mybir.AluOpType.add, op1=mybir.AluOpType.add,
                accum_out=e.stats[:, 2 * gi:2 * gi + 1])
            nc.scalar.activation(out=e.sq_scr[:, :], in_=pt2[:, :],
                                 func=mybir.ActivationFunctionType.Square,
                                 accum_out=e.stats[:, 2 * gi + 1:2 * gi + 2])

    # ---- AllReduce per-token moments across the 8 cores
    cc_in = e.dram.tile([128, 2 * NTILE], f32, name="cc_in")
    cc_out = e.dram.tile([128, 2 * NTILE], f32, name="cc_out")
    nc.sync.dma_start(out=cc_in[:, :], in_=e.stats[:, :])
    nc.gpsimd.collective_compute(
        "AllReduce", mybir.AluOpType.add,
        replica_groups=[list(range(N_CORES))],
        ins=[cc_in.opt()], outs=[cc_out.opt()])
    statsf = e.const.tile([128, 2 * NTILE], f32, tag="statsf", name="statsf")
    nc.sync.dma_start(out=statsf[:, :], in_=cc_out[:, :])
    if e.debug:
        nc.sync.dma_start(out=e.dbg_stats[:, :], in_=e.stats[:, :])
        nc.sync.dma_start(out=e.dbg_statsf[:, :], in_=statsf[:, :])

    # ---- moments -> mean, rstd  [128, 32]
    sf3 = statsf.rearrange("p (i two) -> p i two", two=2)
    mean = e.const.tile([128, NTILE], f32, tag="mean", name="mean")
    rstd = e.const.tile([128, NTILE], f32, tag="rstd", name="rstd")
    var = e.const.tile([128, NTILE], f32, tag="var", name="var")
    msq = e.const.tile([128, NTILE], f32, tag="msq", name="msq")
    nc.vector.tensor_scalar_mul(mean[:, :], sf3[:, :, 0], 1.0 / C)
    nc.vector.tensor_scalar_mul(var[:, :], sf3[:, :, 1], 1.0 / C)
    nc.vector.tensor_mul(msq[:, :], mean[:, :], mean[:, :])
    nc.vector.tensor_sub(var[:, :], var[:, :], msq[:, :])
    nc.scalar.activation(out=var[:, :], in_=var[:, :],
                         func=mybir.ActivationFunctionType.Sqrt,
                         bias=e.eps_t[:, :], scale=1.0)
    nc.vector.reciprocal(rstd[:, :], var[:, :])

    # ---- apply LN + folded (1-lamb)*gamma/beta, store slice
    for gi in range(NTILE):
        o1 = e.sbo.tile([128, CS], f32, tag="o1", name="o1")
        nc.vector.tensor_scalar(
            out=o1[:, :], in0=e.pre3[:, gi, :],
            scalar1=mean[:, gi:gi + 1], scalar2=rstd[:, gi:gi + 1],
            op0=mybir.AluOpType.subtract, op1=mybir.AluOpType.mult)
        o2 = e.sbo.tile([128, CS], f32, tag="o2", name="o2")
        nc.vector.tensor_mul(o2[:, :], o1[:, :], e.gamma[:, :])
        nc.vector.tensor_add(o2[:, :], o2[:, :], e.beta[:, :])
        nc.sync.dma_start(out=e.out_d[gi * 128:(gi + 1) * 128, :], in_=o2[:, :])


def _build(lamb: float):
    f32 = mybir.dt.float32
    mmdt = MM_DTYPE
    nc = bass.Bass(num_devices=N_CORES)
    e = _Env()

    xt_d = nc.declare_dram_parameter("xt", [C, BT], f32, isOutput=False)
    w_d = nc.declare_dram_parameter("wp", [5, C, CS], f32, isOutput=False)
    g_d = nc.declare_dram_parameter("gm", [CS], f32, isOutput=False)
    b_d = nc.declare_dram_parameter("bt", [CS], f32, isOutput=False)
    e.out_d = nc.declare_dram_parameter("out", [BT, CS], f32, isOutput=True)
    e.debug = bool(int(os.environ.get("BASS_DEBUG_DUMPS", "0")))
    if e.debug:
        e.dbg_qk = nc.declare_dram_parameter("dbg_qk", [4, 128, T], f32, isOutput=True)
        e.dbg_vt = nc.declare_dram_parameter("dbg_vt", [128, T], f32, isOutput=True)
        e.dbg_stack = nc.declare_dram_parameter("dbg_stack", [128, T], f32, isOutput=True)
        e.dbg_stats = nc.declare_dram_parameter("dbg_stats", [128, 2 * NTILE], f32, isOutput=True)
        e.dbg_statsf = nc.declare_dram_parameter("dbg_statsf", [128, 2 * NTILE], f32, isOutput=True)

    e.xt3 = xt_d.ap().rearrange("(k p) t -> p k t", p=128)          # [128, 8, 4096]
    w4 = w_d.ap().rearrange("w (k p) m -> w k p m", p=128)          # [5, 8, 128, 128]

    with tile.TileContext(nc) as tc, ExitStack() as ctx:
        e.const = ctx.enter_context(tc.tile_pool(name="const", bufs=1))
        e.sbx = ctx.enter_context(tc.tile_pool(name="sbx", bufs=2))
        e.sbqk = ctx.enter_context(tc.tile_pool(name="sbqk", bufs=1))
        e.sbe = ctx.enter_context(tc.tile_pool(name="sbe", bufs=3))
        e.sbn = ctx.enter_context(tc.tile_pool(name="sbn", bufs=2))
        e.sbo = ctx.enter_context(tc.tile_pool(name="sbo", bufs=2))
        e.ps_a = ctx.enter_context(tc.tile_pool(name="ps_a", bufs=2, space="PSUM"))
        e.ps_s = ctx.enter_context(tc.tile_pool(name="ps_s", bufs=2, space="PSUM"))
        e.ps_o = ctx.enter_context(tc.tile_pool(name="ps_o", bufs=1, space="PSUM"))
        e.dram = ctx.enter_context(tc.tile_pool(name="dram", bufs=1, space="DRAM"))

        # ---- constants ----
        e.ident = e.const.tile([128, 128], f32, tag="ident", name="ident")
        make_identity(nc, e.ident)
        e.gamma = e.const.tile([128, CS], f32, tag="gamma", name="gamma")
        e.beta = e.const.tile([128, CS], f32, tag="beta", name="beta")
        nc.sync.dma_start(out=e.gamma, in_=g_d.ap().partition_broadcast(128))
        nc.sync.dma_start(out=e.beta, in_=b_d.ap().partition_broadcast(128))
        e.eps_t = e.const.tile([128, 1], f32, tag="eps", name="eps_t")
        nc.vector.memset(e.eps_t, EPS)

        # weights: 5 proj x 8 k-tiles, each [128 c, 128 m]
        e.w_sb = []
        for p5 in range(5):
            row = []
            for k in range(8):
                wt = e.const.tile([128, 128], mmdt, tag=f"w{p5}{k}", name=f"w{p5}{k}")
                nc.sync.dma_start(out=wt, in_=w4[p5, k].bitcast(mmdt))
                row.append(wt)
            e.w_sb.append(row)

        # AV stationary tiles [t_k 128, 64 V | 64 ones] per (head, t_k tile)
        e.avw = [[e.const.tile([128, 128], mmdt, tag=f"avw{h}{i}", name=f"avw{h}{i}")
                  for i in range(NT)] for h in range(HPC)]
        ones_t = e.const.tile([128, HS], f32, tag="ones_t", name="ones_t")
        nc.vector.memset(ones_t, 1.0)
        for h in range(HPC):
            for i in range(NT):
                nc.vector.tensor_copy(e.avw[h][i][:, HS:128], ones_t[:, :])

        # persistent per-b projection buffers [128, T]
        e.qk = [e.sbqk.tile([128, T], mmdt, tag=f"qk{w}", name=f"qk{w}")
                for w in range(4)]
        e.vT = e.sbqk.tile([128, T], f32, tag="vT", name="vT")
        e.stack = e.sbqk.tile([128, T], f32, tag="stack", name="stack")
        e.preln = e.sbqk.tile([128, BT], f32, tag="preln", name="preln")
        e.stats = e.const.tile([128, 2 * NTILE], f32, tag="stats", name="stats")
        e.sq_scr = e.const.tile([128, 128], f32, tag="sq_scr", name="sq_scr")
        e.pre3 = e.preln.rearrange("p (i c) -> p i c", c=128)

        nrep = int(os.environ.get("BASS_REPEAT", "1"))
        for _ in range(nrep):
            _emit_compute(nc, e, lamb)

    _legalize_waits(nc)
    return nc


_cache = {}


def _get_nc(lamb: float):
    key = (round(lamb, 9), str(MM_DTYPE),
           os.environ.get("BASS_DEBUG_DUMPS", "0"),
           os.environ.get("BASS_REPEAT", "1"))
    if key not in _cache:
        _cache[key] = _build(lamb)
    return _cache[key]


def kernel(x, wq1, wk1, wq2, wk2, wv, ln_gamma, ln_beta, lamb):
    x = np.asarray(x, dtype=np.float32)
    lam = float(np.asarray(lamb))
    xt = np.ascontiguousarray(x.reshape(BT, C).T)          # [C, BT]
    g = np.asarray(ln_gamma, np.float32) * (1.0 - lam)
    bt = np.asarray(ln_beta, np.float32) * (1.0 - lam)

    nc = _get_nc(lam)
    in_maps = []
    for c in range(N_CORES):
        h0 = c * HPC
        wp = np.stack([
            np.concatenate([np.asarray(w, np.float32)[h0 + j] for j in range(HPC)], axis=1)
            for w in (wq1, wk1, wq2, wk2, wv)
        ])                                                  # [5, C, 128]
        in_maps.append({
            "xt": xt,
            "wp": np.ascontiguousarray(wp),
            "gm": np.ascontiguousarray(g[c * CS:(c + 1) * CS]),
            "bt": np.ascontiguousarray(bt[c * CS:(c + 1) * CS]),
        })

    res = run_bass_kernel_spmd(nc, in_maps, list(range(N_CORES)))
    full = np.concatenate([res.results[c]["out"] for c in range(N_CORES)], axis=1)
    return full.reshape(B, T, C)


# revision 14
# speedup vs baseline: 15.9433x; 1.0771x over previous
"""MultiHeadDifferentialAttention on 8 Trainium2 NeuronCores.

Sharding: tensor-parallel over heads — core c computes heads 2c, 2c+1 for both
batch elements (full attention over T for its heads), producing the channel
slice out[:, :, 128c:128(c+1)] of the pre-LayerNorm concat.  LayerNorm needs
full-channel moments per token, so each core contributes per-token partial
(sum, sum_sq) over its 128 channels and a 32KB AllReduce(add) over all 8 cores
yields the full moments; each core then normalizes its own channel slice.
Host-side: x is pre-transposed to x^T [C, B*T] (the PE contracts along the
partition axis, so x must enter with C on partitions), weights are packed per
head pair, and the final [B*T, 128] slices are concatenated channel-wise.

Attention math per (b, h): out = softmax(q1 k1^T/8) v - lamb*softmax(q2 k2^T/8) v.
Scores are computed transposed (S^T = K Q^T, [t_k, t_q]) so exp(S^T) tiles feed
the AV matmul directly as the moving operand with t_k on partitions (no giant
transposes).  Softmax skips max-subtraction: scores are ~N(0,1) here, so exp is
safe in fp32.  The denominator rides along in the AV matmul: the stationary
operand is [V_h | ones] ([t_k 128, 64+64]), so PSUM rows 0-63 accumulate
(E V)^T and rows 64-127 accumulate sum_tk(E) replicated — the divide is then a
plain lane-wise DVE op.  (1-lamb)*gamma and (1-lamb)*beta are folded host-side.
"""
import os
import numpy as np
from contextlib import ExitStack

import concourse.bass as bass
import concourse.mybir as mybir
import concourse.tile as tile
from concourse.bass_utils import run_bass_kernel_spmd
from concourse.masks import make_identity

N_CORES = 8
B, T, C, H = 2, 2048, 1024, 16
HS = C // H                      # 64
HPC = H // N_CORES               # heads per core = 2
CS = HPC * HS                    # channel slice per core = 128
BT = B * T                       # 4096
NT = T // 128                    # 16 t_k tiles per b
NQ = T // 1024                   # 2 t_q chunks of 1024 per b
NTILE = BT // 128                # 32 output row tiles
EPS = 1e-5

# matmul input dtype: float32r (fast, ~1e-4 rounded) or float32 (exact, 4x slower)
MM_DTYPE = {
    "fp32r": mybir.dt.float32r,
    "fp32": mybir.dt.float32,
}[os.environ.get("BASS_MM_DTYPE", "fp32r")]

_uid = [0]


def _legalize_waits(nc):
    """Split multi-wait instructions into 1-wait NoOps + instruction.

    The walrus build in this container accepts one sync-wait command per
    instruction, but TileContext emits instructions carrying several (notably
    its kernel-tail drain).  Engine-queue instructions execute in order, so
    hoisting extra waits onto same-engine NoOps right before is
    semantics-preserving.
    """
    for fn in nc.m.functions:
        for bb in fn.blocks:
            insts = list(bb.instructions)
            out = []
            changed = False
            for ins in insts:
                si = getattr(ins, "sync_info", None)
                waits = list(si.on_wait) if si is not None and si.on_wait else []
                if len(waits) > 1:
                    changed = True
                    for w in waits[:-1]:
                        _uid[0] += 1
                        out.append(mybir.InstNoOp(
                            name=f"I-waitsplit-{_uid[0]}",
                            sync_info=mybir.SyncInfo(on_wait=[w], on_update=[]),
                            bass_nofuse=True,
                            engine=ins.engine,
                        ))
                    ins.sync_info = mybir.SyncInfo(
                        on_wait=[waits[-1]], on_update=list(si.on_update or [])
                    )
                out.append(ins)
            if changed:
                bb.instructions = out


class _Env:
    pass


def _emit_compute(nc, e, lamb):
    """One full forward pass: projections, attention, LN. Emitted `nrep` times
    for slope-based HW timing (BASS_REPEAT)."""
    f32 = mybir.dt.float32
    mmdt = MM_DTYPE

    for b in range(B):
        # ---- projections: q1,k1,q2,k2 -> qk[w] ([2h*hs, T] transposed), v -> vT
        for ch in range(4):                       # 512-token chunks
            xt_sb = e.sbx.tile([128, 8, 512], mmdt, tag="xt", name="xt_sb")
            col0 = b * T + ch * 512
            nc.sync.dma_start(out=xt_sb, in_=e.xt3[:, :, col0:col0 + 512].bitcast(mmdt))
            for p5 in range(5):
                pp = e.ps_a.tile([128, 512], f32, tag="pp", name="pp")
                for k in range(8):
                    nc.tensor.matmul(pp[:, :], e.w_sb[p5][k][:, :], xt_sb[:, k, :],
                                     start=(k == 0), stop=(k == 7))
                dst = e.qk[p5] if p5 < 4 else e.vT
                nc.vector.tensor_copy(dst[:, ch * 512:(ch + 1) * 512], pp[:, :])

        # ---- V^T -> V tiles into avw[h][i][:, 0:64]
        for i in range(NT):
            pt = e.ps_a.tile([128, 128], f32, tag="pp", name="pt")
            nc.tensor.transpose(pt[:, :], e.vT[:, i * 128:(i + 1) * 128], e.ident[:, :])
            for h in range(HPC):
                nc.vector.tensor_copy(e.avw[h][i][:, 0:HS], pt[:, h * HS:(h + 1) * HS])

        # ---- attention per (h, qc, type)
        for h in range(HPC):
            hp = h * HS
            for qc in range(NQ):
                q0 = qc * 1024
                norm1 = e.sbn.tile([HS, 1024], f32, tag="norm1", name="norm1")
                for ty in range(2):
                    qb, kb = e.qk[2 * ty], e.qk[2 * ty + 1]
                    po = e.ps_o.tile([128, 1024], f32, tag="po", name="po")
                    for tk in range(NT):
                        sS = e.ps_s.tile([128, 1024], f32, tag="sS", name="sS")
                        for hf in range(2):
                            nc.tensor.matmul(
                                sS[:, hf * 512:(hf + 1) * 512],
                                kb[hp:hp + HS, tk * 128:(tk + 1) * 128],
                                qb[hp:hp + HS, q0 + hf * 512:q0 + (hf + 1) * 512],
                                start=True, stop=True)
                        eT = e.sbe.tile([128, 1024], mmdt, tag="eT", name="eT")
                        nc.scalar.activation(out=eT[:, :], in_=sS[:, :],
                                             func=mybir.ActivationFunctionType.Exp,
                                             scale=0.125)
                        for hf in range(2):
                            nc.tensor.matmul(
                                po[:, hf * 512:(hf + 1) * 512],
                                e.avw[h][tk][:, :],
                                eT[:, hf * 512:(hf + 1) * 512],
                                start=(tk == 0), stop=(tk == NT - 1))
                    # normalize: rows 0:64 = (E V)^T, rows 64:128 = den
                    rcp = e.sbn.tile([HS, 1024], f32, tag="rcp", name="rcp")
                    nc.vector.reciprocal(rcp[:, :], po[HS:128, :])
                    if ty == 0:
                        nc.vector.tensor_mul(norm1[:, :], po[0:HS, :], rcp[:, :])
                    else:
                        t2 = e.sbn.tile([HS, 1024], f32, tag="t2", name="t2")
                        nc.vector.tensor_mul(t2[:, :], po[0:HS, :], rcp[:, :])
                        nc.vector.scalar_tensor_tensor(
                            out=e.stack[hp:hp + HS, q0:q0 + 1024],
                            in0=t2[:, :], scalar=-lamb, in1=norm1[:, :],
                            op0=mybir.AluOpType.mult, op1=mybir.AluOpType.add)

        if e.debug and b == 0:
            for w in range(4):
                nc.sync.dma_start(out=e.dbg_qk[w], in_=e.qk[w][:, :].bitcast(f32))
            nc.sync.dma_start(out=e.dbg_vt[:, :], in_=e.vT[:, :])
            nc.sync.dma_start(out=e.dbg_stack[:, :], in_=e.stack[:, :])

        # ---- transpose combined -> [t, chan], moment partials
        for i in range(NT):
            gi = b * NT + i
            pt2 = e.ps_a.tile([128, 128], f32, tag="pp", name="pt2")
            nc.tensor.transpose(pt2[:, :], e.stack[:, i * 128:(i + 1) * 128], e.ident[:, :])
            nc.vector.tensor_scalar(
                out=e.pre3[:, gi, :], in0=pt2[:, :], scalar1=0.0, scalar2=0.0,
                op0=mybir.AluOpType.add, op1=mybir.AluOpType.add,
                accum_out=e.stats[:, 2 * gi:2 * gi + 1])
            nc.scalar.activation(out=e.sq_scr[:, :], in_=pt2[:, :],
                                 func=mybir.ActivationFunctionType.Square,
                                 accum_out=e.stats[:, 2 * gi + 1:2 * gi + 2])

    # ---- AllReduce per-token moments across the 8 cores
    statsf = e.const.tile([128, 2 * NTILE], f32, tag="statsf", name="statsf")
    if os.environ.get("BASS_SKIP_CC", "0") == "1":
        nc.vector.tensor_copy(statsf[:, :], e.stats[:, :])  # timing-only: wrong stats
    else:
        cc_in = e.dram.tile([128, 2 * NTILE], f32, name="cc_in")
        cc_out = e.dram.tile([128, 2 * NTILE], f32, name="cc_out")
        nc.sync.dma_start(out=cc_in[:, :], in_=e.stats[:, :])
        nc.gpsimd.collective_compute(
            "AllReduce", mybir.AluOpType.add,
            replica_groups=[list(range(N_CORES))],
            ins=[cc_in.opt()], outs=[cc_out.opt()])
        nc.sync.dma_start(out=statsf[:, :], in_=cc_out[:, :])
    if e.debug:
        nc.sync.dma_start(out=e.dbg_stats[:, :], in_=e.stats[:, :])
        nc.sync.dma_start(out=e.dbg_statsf[:, :], in_=statsf[:, :])

    # ---- moments -> mean, rstd  [128, 32]
    sf3 = statsf.rearrange("p (i two) -> p i two", two=2)
    mean = e.const.tile([128, NTILE], f32, tag="mean", name="mean")
    rstd = e.const.tile([128, NTILE], f32, tag="rstd", name="rstd")
    var = e.const.tile([128, NTILE], f32, tag="var", name="var")
    msq = e.const.tile([128, NTILE], f32, tag="msq", name="msq")
    nc.vector.tensor_scalar_mul(mean[:, :], sf3[:, :, 0], 1.0 / C)
    nc.vector.tensor_scalar_mul(var[:, :], sf3[:, :, 1], 1.0 / C)
    nc.vector.tensor_mul(msq[:, :], mean[:, :], mean[:, :])
    nc.vector.tensor_sub(var[:, :], var[:, :], msq[:, :])
    nc.scalar.activation(out=var[:, :], in_=var[:, :],
                         func=mybir.ActivationFunctionType.Sqrt,
                         bias=e.eps_t[:, :], scale=1.0)
    nc.vector.reciprocal(rstd[:, :], var[:, :])

    # ---- apply LN + folded (1-lamb)*gamma/beta, store slice
    for gi in range(NTILE):
        o1 = e.sbo.tile([128, CS], f32, tag="o1", name="o1")
        nc.vector.tensor_scalar(
            out=o1[:, :], in0=e.pre3[:, gi, :],
            scalar1=mean[:, gi:gi + 1], scalar2=rstd[:, gi:gi + 1],
            op0=mybir.AluOpType.subtract, op1=mybir.AluOpType.mult)
        o2 = e.sbo.tile([128, CS], f32, tag="o2", name="o2")
        nc.vector.tensor_mul(o2[:, :], o1[:, :], e.gamma[:, :])
        nc.vector.tensor_add(o2[:, :], o2[:, :], e.beta[:, :])
        nc.sync.dma_start(out=e.out_d[gi * 128:(gi + 1) * 128, :], in_=o2[:, :])


def _build(lamb: float):
    f32 = mybir.dt.float32
    mmdt = MM_DTYPE
    nc = bass.Bass(num_devices=N_CORES)
    e = _Env()

    xt_d = nc.declare_dram_parameter("xt", [C, BT], f32, isOutput=False)
    w_d = nc.declare_dram_parameter("wp", [5, C, CS], f32, isOutput=False)
    g_d = nc.declare_dram_parameter("gm", [CS], f32, isOutput=False)
    b_d = nc.declare_dram_parameter("bt", [CS], f32, isOutput=False)
    e.out_d = nc.declare_dram_parameter("out", [BT, CS], f32, isOutput=True)
    e.debug = bool(int(os.environ.get("BASS_DEBUG_DUMPS", "0")))
    if e.debug:
        e.dbg_qk = nc.declare_dram_parameter("dbg_qk", [4, 128, T], f32, isOutput=True)
        e.dbg_vt = nc.declare_dram_parameter("dbg_vt", [128, T], f32, isOutput=True)
        e.dbg_stack = nc.declare_dram_parameter("dbg_stack", [128, T], f32, isOutput=True)
        e.dbg_stats = nc.declare_dram_parameter("dbg_stats", [128, 2 * NTILE], f32, isOutput=True)
        e.dbg_statsf = nc.declare_dram_parameter("dbg_statsf", [128, 2 * NTILE], f32, isOutput=True)

    e.xt3 = xt_d.ap().rearrange("(k p) t -> p k t", p=128)          # [128, 8, 4096]
    w4 = w_d.ap().rearrange("w (k p) m -> w k p m", p=128)          # [5, 8, 128, 128]

    with tile.TileContext(nc) as tc, ExitStack() as ctx:
        e.const = ctx.enter_context(tc.tile_pool(name="const", bufs=1))
        e.sbx = ctx.enter_context(tc.tile_pool(name="sbx", bufs=2))
        e.sbqk = ctx.enter_context(tc.tile_pool(name="sbqk", bufs=1))
        e.sbe = ctx.enter_context(tc.tile_pool(name="sbe", bufs=3))
        e.sbn = ctx.enter_context(tc.tile_pool(name="sbn", bufs=2))
        e.sbo = ctx.enter_context(tc.tile_pool(name="sbo", bufs=2))
        e.ps_a = ctx.enter_context(tc.tile_pool(name="ps_a", bufs=2, space="PSUM"))
        e.ps_s = ctx.enter_context(tc.tile_pool(name="ps_s", bufs=2, space="PSUM"))
        e.ps_o = ctx.enter_context(tc.tile_pool(name="ps_o", bufs=1, space="PSUM"))
        e.dram = ctx.enter_context(tc.tile_pool(name="dram", bufs=1, space="DRAM"))

        # ---- constants ----
        e.ident = e.const.tile([128, 128], f32, tag="ident", name="ident")
        make_identity(nc, e.ident)
        e.gamma = e.const.tile([128, CS], f32, tag="gamma", name="gamma")
        e.beta = e.const.tile([128, CS], f32, tag="beta", name="beta")
        nc.sync.dma_start(out=e.gamma, in_=g_d.ap().partition_broadcast(128))
        nc.sync.dma_start(out=e.beta, in_=b_d.ap().partition_broadcast(128))
        e.eps_t = e.const.tile([128, 1], f32, tag="eps", name="eps_t")
        nc.vector.memset(e.eps_t, EPS)

        # weights: 5 proj x 8 k-tiles, each [128 c, 128 m]
        e.w_sb = []
        for p5 in range(5):
            row = []
            for k in range(8):
                wt = e.const.tile([128, 128], mmdt, tag=f"w{p5}{k}", name=f"w{p5}{k}")
                nc.sync.dma_start(out=wt, in_=w4[p5, k].bitcast(mmdt))
                row.append(wt)
            e.w_sb.append(row)

        # AV stationary tiles [t_k 128, 64 V | 64 ones] per (head, t_k tile)
        e.avw = [[e.const.tile([128, 128], mmdt, tag=f"avw{h}{i}", name=f"avw{h}{i}")
                  for i in range(NT)] for h in range(HPC)]
        ones_t = e.const.tile([128, HS], f32, tag="ones_t", name="ones_t")
        nc.vector.memset(ones_t, 1.0)
        for h in range(HPC):
            for i in range(NT):
                nc.vector.tensor_copy(e.avw[h][i][:, HS:128], ones_t[:, :])

        # persistent per-b projection buffers [128, T]
        e.qk = [e.sbqk.tile([128, T], mmdt, tag=f"qk{w}", name=f"qk{w}")
                for w in range(4)]
        e.vT = e.sbqk.tile([128, T], f32, tag="vT", name="vT")
        e.stack = e.sbqk.tile([128, T], f32, tag="stack", name="stack")
        e.preln = e.sbqk.tile([128, BT], f32, tag="preln", name="preln")
        e.stats = e.const.tile([128, 2 * NTILE], f32, tag="stats", name="stats")
        e.sq_scr = e.const.tile([128, 128], f32, tag="sq_scr", name="sq_scr")
        e.pre3 = e.preln.rearrange("p (i c) -> p i c", c=128)

        nrep = int(os.environ.get("BASS_REPEAT", "1"))
        for _ in range(nrep):
            _emit_compute(nc, e, lamb)

    _legalize_waits(nc)
    return nc


_cache = {}


def _get_nc(lamb: float):
    key = (round(lamb, 9), str(MM_DTYPE),
           os.environ.get("BASS_DEBUG_DUMPS", "0"),
           os.environ.get("BASS_REPEAT", "1"),
           os.environ.get("BASS_SKIP_CC", "0"))
    if key not in _cache:
        _cache[key] = _build(lamb)
    return _cache[key]


def kernel(x, wq1, wk1, wq2, wk2, wv, ln_gamma, ln_beta, lamb):
    x = np.asarray(x, dtype=np.float32)
    lam = float(np.asarray(lamb))
    xt = np.ascontiguousarray(x.reshape(BT, C).T)          # [C, BT]
    g = np.asarray(ln_gamma, np.float32) * (1.0 - lam)
    bt = np.asarray(ln_beta, np.float32) * (1.0 - lam)

    nc = _get_nc(lam)
    in_maps = []
    for c in range(N_CORES):
        h0 = c * HPC
        wp = np.stack([
            np.concatenate([np.asarray(w, np.float32)[h0 + j] for j in range(HPC)], axis=1)
            for w in (wq1, wk1, wq2, wk2, wv)
        ])                                                  # [5, C, 128]
        in_maps.append({
            "xt": xt,
            "wp": np.ascontiguousarray(wp),
            "gm": np.ascontiguousarray(g[c * CS:(c + 1) * CS]),
            "bt": np.ascontiguousarray(bt[c * CS:(c + 1) * CS]),
        })

    res = run_bass_kernel_spmd(nc, in_maps, list(range(N_CORES)))
    full = np.concatenate([res.results[c]["out"] for c in range(N_CORES)], axis=1)
    return full.reshape(B, T, C)
